# revision 1
# baseline (speedup 1.0000x reference)
# Trainium2 Bass kernel for an 8-layer dense transformer (B=4, T=1024,
# V=E=1024, H=16, M=4096), 8-way SPMD across one chip.
#
# Sharding: data-parallel over (batch x 2 interleaved token chunks) ->
# 8 shards of 512 tokens.  Even cores own logical 256-token chunks (0,3)
# of their batch, odd cores own (1,2), which balances causal-attention
# work.  Attention needs the full batch's K/V, which lives on exactly
# one partner core, so K and V are exchanged through PAIR-group
# AllGathers ([[0,1],[2,3],...]) -- 1/7th the traffic of an 8-rank
# gather.  Everything else runs locally with replicated weights.
#
# Layouts: the residual stream is feature-major ([E on partitions,
# tokens free]) so every GEMM consumes the natural row-major weight
# layout with zero transposes.  Scores are key-major so the PV matmul
# is native; the softmax denominator comes out of the same PV matmul
# via a 65th all-ones column appended to V; softmax skips the max
# subtraction (logits are O(1) here).  Causal masking is a per-core
# 0/1 multiplicative bf16 mask so the program is identical on all
# cores; cross-partition LN stats/broadcasts go through tiny
# ones-matmuls on the tensor engine, with the LN gain/bias folded into
# rank-2 outer-product broadcasts.
#
# All GEMM operands are bf16 (weights pre-rounded on the host), which
# halves the weight HBM stream vs fp32 and runs the PE at full rate.
# The whole layer's QKV weights are preloaded into SBUF during the
# previous layer's MLP so the projection phase never waits on HBM, and
# the K allgather is issued right after the K projection so it flies
# under the Q/V projections.

import os
import sys

for _p in ("/opt/trn_rl_repo", "/root/.axon_site/_ro/trn_rl_repo"):
    if _p not in sys.path and os.path.isdir(_p):
        sys.path.insert(0, _p)

import numpy as np

import concourse.bass as bass
import concourse.mybir as mybir
import concourse.tile as tile
from concourse import bacc
from concourse.bass_utils import run_bass_kernel_spmd

B, T, V, E, H, M, L, C = 4, 1024, 1024, 1024, 16, 4096, 8, 64
NC = 8          # cores
S = 512         # tokens per core
CH = 256        # chunk size
EPS = 1e-5
SCALE = 1.0 / 8.0   # 1/sqrt(C)

F32 = mybir.dt.float32
BF16 = mybir.dt.bfloat16
AOT = mybir.AluOpType
AFT = mybir.ActivationFunctionType

N_LAYERS = int(os.environ.get("KERNEL_LAYERS", str(L)))
DEBUG_X = bool(int(os.environ.get("KERNEL_DEBUG_X", "0")))
# Replace collectives with local DMA copies (timing experiments).
FAKE_AG = bool(int(os.environ.get("KERNEL_FAKE_AG", "0")))
# Build single-core (offline timeline-simulation only).
SINGLE = bool(int(os.environ.get("KERNEL_SINGLE", "0")))
# Repeat the computation R times in a hardware loop (timing).
HWLOOP = int(os.environ.get("KERNEL_HWLOOP", "1"))
if SINGLE or HWLOOP > 1:
    FAKE_AG = True

# Pair-group allgather concat order is [even rank, odd rank], so the
# gathered K/V token order is [even core's (0,3) | odd core's (1,2)]:
# gathered slot g -> logical chunk GORDER[g].
GORDER = [0, 3, 1, 2]
# Core parity -> logical chunks of its two local q-slots.
QCH = [(0, 3), (1, 2)]

KBLOB = S * E             # elems per rank K blob (bf16), [8*128, 512]
VBLOB = S * E             # elems per rank V blob (bf16), [512, 1024]

# P-tile column base for each (g, kt) score block.
PCOL = {(0, 0): 0, (0, 1): 512, (2, 0): 1024, (2, 1): 1536,
        (1, 0): 2048, (1, 1): 2304, (3, 0): 2560, (3, 1): 2816}
BLK512 = [(0, 0), (0, 1), (2, 0), (2, 1)]   # N=512 (both q slots)
BLK256 = [(1, 0), (1, 1), (3, 0), (3, 1)]   # N=256 (q slot 1 only)
# PV accumulation order: first/last must be full-width (N=512) blocks.
PV_ORDER = [(0, 0), (1, 0), (1, 1), (3, 0), (3, 1), (0, 1), (2, 0), (2, 1)]


def build_program():
    nc = bacc.Bacc("TRN2", target_bir_lowering=False, debug=False,
                   num_devices=(1 if SINGLE else NC))

    # ---- DRAM I/O ----
    d_toksT = nc.dram_tensor("toksT", [V, S], BF16, kind="ExternalInput")
    d_posT = nc.dram_tensor("posT", [E, S], F32, kind="ExternalInput")
    d_maska = nc.dram_tensor("maska", [128, 4, CH], BF16, kind="ExternalInput")
    d_maskb = nc.dram_tensor("maskb", [128, 4 * CH], BF16, kind="ExternalInput")
    d_reps = (nc.dram_tensor("reps", [1, 1], mybir.dt.uint32,
                             kind="ExternalInput") if HWLOOP > 1 else None)
    d_wqk = nc.dram_tensor("wqk", [L, E, H // 2, 4 * C], BF16, kind="ExternalInput")
    d_wv = nc.dram_tensor("wv", [L, E, H * C], BF16, kind="ExternalInput")
    d_w1 = nc.dram_tensor("w1", [L, E, M], BF16, kind="ExternalInput")
    d_w2 = nc.dram_tensor("w2", [L, M, E], BF16, kind="ExternalInput")
    d_b1 = nc.dram_tensor("b1", [L, M], F32, kind="ExternalInput")
    d_b2 = nc.dram_tensor("b2", [L, 128, 8], F32, kind="ExternalInput")
    d_gb1 = nc.dram_tensor("gb1", [L, 2, E], BF16, kind="ExternalInput")
    d_gb2 = nc.dram_tensor("gb2", [L, 2, E], BF16, kind="ExternalInput")
    d_gbf = nc.dram_tensor("gbf", [2, E], BF16, kind="ExternalInput")
    d_tokw = nc.dram_tensor("tokw", [V, E], BF16, kind="ExternalInput")
    d_uw = nc.dram_tensor("uw", [E, V], BF16, kind="ExternalInput")
    d_ub = nc.dram_tensor("ub", [128, 8], F32, kind="ExternalInput")
    d_out = nc.dram_tensor("logits", [V, S], F32, kind="ExternalOutput")

    with tile.TileContext(nc) as tc:
        from contextlib import ExitStack
        top = ExitStack()
        pers = top.enter_context(tc.tile_pool(name="pers", bufs=1))
        small = top.enter_context(tc.tile_pool(name="small", bufs=1))
        stage = top.enter_context(tc.tile_pool(name="stage", bufs=2))
        bigpool = top.enter_context(tc.tile_pool(name="bigpool", bufs=1))
        dram = top.enter_context(tc.tile_pool(name="dram", bufs=2, space="DRAM"))
        dram_sh = top.enter_context(tc.tile_pool(name="dram_sh", bufs=2,
                                                 space="DRAM"))

        # ---- persistent SBUF state ----
        x = pers.tile([128, 8, S], F32)        # residual (feature-major)
        inner = pers.tile([128, 8, S], F32)    # x + attn_out (+ mlp out)
        h = pers.tile([128, 8, S], BF16)       # LN output / rounded x
        sq = pers.tile([128, 8, S], BF16)      # squares for LN stats
        qsb = pers.tile([128, 8, S], BF16)     # Q (head-pair-major)
        wqk_sb = pers.tile([128, 8, H // 2, 4 * C], BF16)   # layer QKV weights
        wv_sb = pers.tile([128, 8, H * C], BF16)
        maska = pers.tile([128, 4, CH], BF16)
        maskb = pers.tile([128, 4 * CH], BF16)
        ones_col = pers.tile([128, 1], BF16)   # stats lhsT, carries 1/E
        ones_row = pers.tile([1, 128], BF16)   # denom broadcast lhsT
        eps_col = pers.tile([1, 1], BF16)      # +EPS via stats matmul
        ones_s = pers.tile([1, S], BF16)
        gb1 = pers.tile([2, E], BF16)
        gb2 = pers.tile([2, E], BF16)
        gbf = pers.tile([2, E], BF16)
        b1sb = pers.tile([128, 32], F32)
        b2sb = pers.tile([128, 8], F32)
        ubsb = pers.tile([128, 8], F32)
        lnst = small.tile([1, 4 * S], F32)     # mu | mu2 | var | rvar
        rstd_t = small.tile([1, S], BF16)
        nmu_ones = small.tile([2, S], BF16)    # row0: -mu*rstd, row1: 1.0

        nc.vector.memset(ones_col[:], 1.0 / E)
        nc.vector.memset(ones_row[:], 1.0)
        nc.vector.memset(eps_col[:], EPS)
        nc.vector.memset(ones_s[:], 1.0)
        # both rows 1.0; each layer_norm overwrites row0 with -mu*rstd
        # (engines cannot address partition base 1, so never write row1)
        nc.vector.memset(nmu_ones[:], 1.0)
        nc.sync.dma_start(maska[:], d_maska[:])
        nc.sync.dma_start(maskb[:], d_maskb[:])
        nc.sync.dma_start(gbf[:], d_gbf[:])
        nc.sync.dma_start(ubsb[:], d_ub[:])

        def ln_prep(src_t, t):
            """Stage tile t of src for LN stats: h = bf16 copy, sq = square.
            Called from the producer loop right after src[:, t, :] is
            written, so the stats matmuls can fire immediately."""
            nc.vector.tensor_copy(h[:, t, :], src_t[:, t, :])
            nc.gpsimd.tensor_tensor(sq[:, t, :], src_t[:, t, :],
                                    src_t[:, t, :], AOT.mult)

        def layer_norm(src_t, gb_t):
            """src_t: [128,8,S] F32 sbuf -> h (BF16).  ln_prep must have
            run for all 8 tiles (h holds the bf16 copy, sq the squares).
            h = src*outer(g,rstd) + outer(g,-mu*rstd) + outer(b,1)."""
            nmu = nmu_ones[0:1, :]
            with tc.tile_pool(name="ps_ln", bufs=4, space="PSUM") as ps_ln:
                # two half-width pipelines with separate PSUM stat tiles:
                # the PE fills one half's reduce-chain latency with the
                # other half's stats matmuls
                for ci, cs in enumerate((slice(0, 256), slice(256, 512))):
                    o = ci * 256
                    mu = lnst[:, o:o + 256]
                    mu2 = lnst[:, S + o:S + o + 256]
                    var = lnst[:, 2 * S + o:2 * S + o + 256]
                    rvar = lnst[:, 3 * S + o:3 * S + o + 256]
                    st_mu = ps_ln.tile([1, 256], F32, tag="stat", bufs=4)
                    st_msq = ps_ln.tile([1, 256], F32, tag="stat", bufs=4)
                    for t in range(8):
                        nc.tensor.matmul(st_mu[:], ones_col[:], h[:, t, cs],
                                         start=(t == 0), stop=(t == 7))
                    for t in range(8):
                        nc.tensor.matmul(st_msq[:], ones_col[:], sq[:, t, cs],
                                         start=(t == 0), stop=False)
                    # st_msq += EPS, so var already carries the epsilon
                    nc.tensor.matmul(st_msq[:], eps_col[:], ones_s[:, cs],
                                     start=False, stop=True)
                    nc.scalar.square(mu2, st_mu[:])
                    nc.vector.tensor_tensor(var, st_msq[:], mu2, AOT.subtract)
                    nc.vector.reciprocal(rvar, var)
                    with nc.allow_low_precision(reason="bf16 rstd"):
                        nc.scalar.sqrt(rstd_t[:, cs], rvar)
                        # nmu = (mu * -1) * rstd
                        nc.vector.scalar_tensor_tensor(
                            nmu[:, cs], st_mu[:], -1.0, rstd_t[:, cs],
                            AOT.mult, AOT.mult)
                for t in range(8):
                    bc1 = ps_ln.tile([128, S], F32, tag="bc", bufs=4)
                    bc2 = ps_ln.tile([128, S], F32, tag="bc", bufs=4)
                    gsl = gb_t[0:1, t * 128:(t + 1) * 128]
                    gbsl = gb_t[0:2, t * 128:(t + 1) * 128]
                    nc.tensor.matmul(bc1[:], gsl, rstd_t[:], start=True,
                                     stop=True)
                    nc.tensor.matmul(bc2[:], gbsl, nmu_ones[:], start=True,
                                     stop=True)
                    if t < 5:
                        tmp = stage.tile([128, S], F32, tag="lntmp", bufs=4)
                        nc.vector.tensor_tensor(tmp[:], src_t[:, t, :],
                                                bc1[:], AOT.mult)
                        nc.vector.tensor_tensor(h[:, t, :], tmp[:], bc2[:],
                                                AOT.add)
                    else:
                        # gpsimd cannot read PSUM: stage the broadcasts to
                        # SBUF through the (idle) Act engine, then run the
                        # normalize on the Pool engine alongside the DVE lane
                        bc1c = stage.tile([128, S], BF16, tag="bcc", bufs=4)
                        bc2c = stage.tile([128, S], BF16, tag="bcc2", bufs=4)
                        nc.scalar.copy(bc1c[:], bc1[:])
                        nc.scalar.copy(bc2c[:], bc2[:])
                        tmp = stage.tile([128, S], F32, tag="lntmp", bufs=4)
                        nc.gpsimd.tensor_tensor(tmp[:], src_t[:, t, :],
                                                bc1c[:], AOT.mult)
                        nc.gpsimd.tensor_tensor(h[:, t, :], tmp[:], bc2c[:],
                                                AOT.add)

        def load_qkv_weights(l):
            for e in range(8):
                nc.sync.dma_start(wqk_sb[:, e, :, :],
                                  d_wqk[l, e * 128:(e + 1) * 128, :, :])
                nc.sync.dma_start(wv_sb[:, e, :],
                                  d_wv[l, e * 128:(e + 1) * 128, :])

        def body():
            # ---------------- embedding ----------------
            with tc.tile_pool(name="emb", bufs=1) as emb, \
                 tc.tile_pool(name="wemb", bufs=2) as wemb, \
                 tc.tile_pool(name="ps_emb", bufs=8, space="PSUM") as ps_emb:
                toksr = emb.tile([128, 8, S], BF16)
                psl = [ps_emb.tile([128, S], F32, tag="embps", bufs=8,
                                   name=f"embps{_i}") for _i in range(8)]
                for v in range(8):
                    nc.sync.dma_start(toksr[:, v, :],
                                      d_toksT[v * 128:(v + 1) * 128, :])
                    tw = wemb.tile([128, 8, 128], BF16, tag="twt", bufs=3)
                    nc.sync.dma_start(tw[:],
                                      d_tokw[v * 128:(v + 1) * 128, :]
                                      .rearrange("p (e c) -> p e c", c=128))
                    for e in range(8):
                        nc.tensor.matmul(psl[e][:], tw[:, e, :],
                                         toksr[:, v, :],
                                         start=(v == 0), stop=(v == 7))
                for e in range(8):
                    nc.scalar.copy(x[:, e, :], psl[e][:])
            with tc.tile_pool(name="embp", bufs=1) as embp:
                possb = embp.tile([128, 8, S], F32)
                for e in range(8):
                    nc.sync.dma_start(possb[:, e, :],
                                      d_posT[e * 128:(e + 1) * 128, :])
                    nc.vector.tensor_tensor(x[:, e, :], x[:, e, :],
                                            possb[:, e, :], AOT.add)
                    ln_prep(x, e)
            load_qkv_weights(0)

            # ---------------- layers ----------------
            for l in range(N_LAYERS):
                nc.sync.dma_start(gb1[:], d_gb1[l])
                nc.sync.dma_start(gb2[:], d_gb2[l])
                nc.sync.dma_start(b1sb[:],
                                  d_b1[l].rearrange("(o p) -> p o", p=128))
                nc.sync.dma_start(b2sb[:], d_b2[l])

                agk_in = dram.tile([E, S], BF16, tag="agkin")
                agv_in = dram.tile([S, E], BF16, tag="agvin")
                # pair-group collectives don't support the Shared-output
                # optimization (needs >4 cores) -- Local output is standard
                agk_out = dram_sh.tile([2 * KBLOB], BF16, tag="agkout",
                                       addr_space="Local")
                agv_out = dram_sh.tile([2 * VBLOB], BF16, tag="agvout",
                                       addr_space="Local")

                # ---- LN1 ----
                layer_norm(x, gb1)

                # ---- K projection first; its pair-allgather flies under
                # the Q and V projections ----
                with tc.tile_pool(name="ps_qkv", bufs=8, space="PSUM") as ps_qkv:
                    for hp in range(8):
                        kps = ps_qkv.tile([128, S], F32, tag="qkps", bufs=4)
                        for e in range(8):
                            nc.tensor.matmul(
                                kps[:],
                                wqk_sb[:, e, hp, 128:256],
                                h[:, e, :], start=(e == 0), stop=(e == 7))
                        kst = stage.tile([128, S], BF16, tag="kst", bufs=8)
                        nc.vector.tensor_copy(kst[:], kps[:])
                        nc.sync.dma_start(
                            agk_in[hp * 128:(hp + 1) * 128, :], kst[:])
                    if FAKE_AG:
                        nc.sync.dma_start(
                            agk_out[0:KBLOB]
                            .rearrange("(r c) -> r c", c=S)[0:128, :],
                            agk_in[0:128, :])
                        nc.sync.dma_start(
                            agk_out[KBLOB:2 * KBLOB]
                            .rearrange("(r c) -> r c", c=S)[0:128, :],
                            agk_in[0:128, :])
                    else:
                        nc.gpsimd.collective_compute(
                            "AllGather", AOT.bypass,
                            replica_groups=[[2 * i, 2 * i + 1]
                                            for i in range(NC // 2)],
                            ins=[agk_in[:].rearrange("r c -> (r c)").opt()],
                            outs=[agk_out[:].opt()])
                    # ---- Q projection ----
                    for hp in range(8):
                        qps = ps_qkv.tile([128, S], F32, tag="qkps", bufs=4)
                        for e in range(8):
                            nc.tensor.matmul(
                                qps[:],
                                wqk_sb[:, e, hp, 0:128],
                                h[:, e, :], start=(e == 0), stop=(e == 7))
                        nc.vector.tensor_copy(qsb[:, hp, :], qps[:])
                    # ---- V projection (token-major) ----
                    for tt in range(4):
                        for hf in range(2):
                            vps = ps_qkv.tile([128, S], F32, tag="qkps",
                                              bufs=4)
                            for e in range(8):
                                nc.tensor.matmul(
                                    vps[:],
                                    h[:, e, tt * 128:(tt + 1) * 128],
                                    wv_sb[:, e, hf * 512:(hf + 1) * 512],
                                    start=(e == 0), stop=(e == 7))
                            vst = stage.tile([128, S], BF16, tag="vst",
                                             bufs=8)
                            nc.vector.tensor_copy(vst[:], vps[:])
                            nc.sync.dma_start(
                                agv_in[tt * 128:(tt + 1) * 128,
                                       hf * 512:(hf + 1) * 512], vst[:])
                    if FAKE_AG:
                        nc.sync.dma_start(
                            agv_out[0:VBLOB]
                            .rearrange("(r c) -> r c", c=E)[0:64, :],
                            agv_in[0:64, :])
                        nc.sync.dma_start(
                            agv_out[VBLOB:2 * VBLOB]
                            .rearrange("(r c) -> r c", c=E)[0:64, :],
                            agv_in[0:64, :])
                    else:
                        nc.gpsimd.collective_compute(
                            "AllGather", AOT.bypass,
                            replica_groups=[[2 * i, 2 * i + 1]
                                            for i in range(NC // 2)],
                            ins=[agv_in[:].rearrange("r c -> (r c)").opt()],
                            outs=[agv_out[:].opt()])

                # gathered V (token-major, with interleaved 1/1 column for
                # the softmax denominator)
                vgr = bigpool.tile([128, 8, H, C + 1], BF16, tag="vgrm",
                                   bufs=1)
                for kt in range(8):
                    off = (0 if kt < 4 else VBLOB) + (kt % 4) * 128 * E
                    vsrc = agv_out[bass.ds(off, 128 * E)].rearrange(
                        "(r hh cc) -> r hh cc", hh=H, cc=C)
                    nc.sync.dma_start(vgr[:, kt, :, 0:C], vsrc)
                nc.vector.memset(vgr[:, :, :, C:C + 1], 1.0)

                # ---- attention ----
                with tc.tile_pool(name="attnp", bufs=1) as attnp, \
                     tc.tile_pool(name="ps_s", bufs=4, space="PSUM") as ps_s, \
                     tc.tile_pool(name="ps_y", bufs=2, space="PSUM") as ps_y, \
                     tc.tile_pool(name="ps_by", bufs=2, space="PSUM") as ps_by:
                    for hp in range(8):
                        kr = attnp.tile([128, 1024], BF16, tag="kr", bufs=4)
                        nc.sync.dma_start(
                            kr[:, 0:512],
                            agk_out[bass.ds(hp * 128 * S, 128 * S)]
                            .rearrange("(r c) -> r c", c=S))
                        nc.sync.dma_start(
                            kr[:, 512:1024],
                            agk_out[bass.ds(KBLOB + hp * 128 * S, 128 * S)]
                            .rearrange("(r c) -> r c", c=S))

                        for head in range(2):
                            ha = hp * 2 + head
                            qb = head * 64
                            P = attnp.tile([128, 3072], BF16, tag="P", bufs=2)
                            for (g, kt) in BLK512:
                                ktg = g * 2 + kt
                                sps = ps_s.tile([128, 512], F32, tag="S",
                                                bufs=4)
                                nc.tensor.matmul(
                                    sps[:],
                                    kr[qb:qb + 64,
                                       ktg * 128:(ktg + 1) * 128],
                                    qsb[qb:qb + 64, hp, :],
                                    start=True, stop=True)
                                pc = PCOL[(g, kt)]
                                nc.scalar.activation(P[:, pc:pc + 512], sps[:],
                                                     AFT.Exp, scale=SCALE)
                            for g in (1, 3):
                                sps = ps_s.tile([128, 512], F32, tag="S",
                                                bufs=4)
                                for kt in range(2):
                                    ktg = g * 2 + kt
                                    nc.tensor.matmul(
                                        sps[:, kt * 256:(kt + 1) * 256],
                                        kr[qb:qb + 64,
                                           ktg * 128:(ktg + 1) * 128],
                                        qsb[qb:qb + 64, hp, 256:512],
                                        start=True, stop=True)
                                pc = PCOL[(g, 0)]
                                nc.scalar.activation(P[:, pc:pc + 512], sps[:],
                                                     AFT.Exp, scale=SCALE)
                            # causal mask (0/1 multiplicative)
                            ap1 = P[:, 0:2048].rearrange(
                                "p (b q) -> p b q", q=512)[:, :, 0:CH]
                            nc.vector.tensor_tensor(ap1, ap1, maska[:],
                                                    AOT.mult)
                            ap2 = P[:, 2048:3072]
                            nc.vector.tensor_tensor(ap2, ap2, maskb[:],
                                                    AOT.mult)
                            # PV (+ denominator via ones column)
                            yps = ps_y.tile([128, 512], F32, tag="y", bufs=2)
                            for i, (g, kt) in enumerate(PV_ORDER):
                                ktg = g * 2 + kt
                                pc = PCOL[(g, kt)]
                                n = 512 if (g, kt) in BLK512 else 256
                                qoff = 0 if n == 512 else 256
                                nc.tensor.matmul(
                                    yps[0:65, qoff:qoff + n],
                                    vgr[:, ktg, ha, :],
                                    P[:, pc:pc + n],
                                    start=(i == 0),
                                    stop=(i == len(PV_ORDER) - 1))
                            rd = stage.tile([1, S], BF16, tag="rd", bufs=3)
                            with nc.allow_low_precision(reason="1/d bcast"):
                                nc.vector.reciprocal(rd[:], yps[64:65, :])
                            bcd = ps_by.tile([64, S], F32, tag="bcd", bufs=2)
                            nc.tensor.matmul(bcd[:], ones_row[:, 0:64],
                                             rd[:], start=True, stop=True)
                            ebase = head * 64
                            ysb = stage.tile([128, S], F32, tag="ysb",
                                             bufs=2)
                            ysl = ysb[ebase:ebase + 64, :]
                            nc.vector.tensor_copy(ysl, yps[0:64, :])
                            nc.vector.tensor_tensor(ysl, ysl, bcd[:],
                                                    AOT.mult)
                            # inner = x + y   (e-tile == hp)
                            nc.gpsimd.tensor_tensor(
                                inner[ebase:ebase + 64, hp, :],
                                x[ebase:ebase + 64, hp, :], ysl, AOT.add)
                        ln_prep(inner, hp)
                # ---- LN2 + MLP (mlp out accumulates into `inner`) ----
                layer_norm(inner, gb2)
                # next layer's QKV weights stream during the MLP (the
                # attention window's DMA belongs to the kr/vgr loads)
                if l + 1 < N_LAYERS:
                    load_qkv_weights(l + 1)
                with tc.tile_pool(name="wmlp", bufs=1) as wmlp, \
                     tc.tile_pool(name="ps_m", bufs=4, space="PSUM") as ps_m, \
                     tc.tile_pool(name="ps_o", bufs=4, space="PSUM") as ps_o:
                    for half in range(2):
                        m_sb = bigpool.tile([128, 16, 512], BF16, tag="vgrm",
                                            bufs=1)
                        for og4 in range(4):
                            og = half * 4 + og4
                            mps_l = [ps_m.tile([128, S], F32, tag="mps",
                                               bufs=4, name=f"mps{_i}")
                                     for _i in range(4)]
                            for eh in range(2):
                                w1h = wmlp.tile([128, 4, 512], BF16,
                                                tag="w1h", bufs=2)
                                for e4 in range(4):
                                    e = eh * 4 + e4
                                    nc.sync.dma_start(
                                        w1h[:, e4, :],
                                        d_w1[l, e * 128:(e + 1) * 128,
                                             og * 512:(og + 1) * 512])
                                for ob in range(4):
                                    for e4 in range(4):
                                        e = eh * 4 + e4
                                        nc.tensor.matmul(
                                            mps_l[ob][:],
                                            w1h[:, e4,
                                                ob * 128:(ob + 1) * 128],
                                            h[:, e, :],
                                            start=(e == 0), stop=(e == 7))
                            for ob in range(4):
                                mtl = og4 * 4 + ob
                                mt_abs = half * 16 + mtl
                                nc.scalar.activation(
                                    m_sb[:, mtl, :], mps_l[ob][:], AFT.Relu,
                                    bias=b1sb[:, mt_abs:mt_abs + 1], scale=1.0)
                        for eoh in range(2):
                            ops = [ps_o.tile([128, S], F32, tag="ops", bufs=4,
                                             name=f"ops{_i}")
                                   for _i in range(4)]
                            for mt in range(16):
                                mt_abs = half * 16 + mt
                                w2t = wmlp.tile([128, 512], BF16, tag="w2",
                                                bufs=6)
                                nc.sync.dma_start(
                                    w2t[:],
                                    d_w2[l, mt_abs * 128:(mt_abs + 1) * 128,
                                         eoh * 512:(eoh + 1) * 512])
                                for eo4 in range(4):
                                    nc.tensor.matmul(
                                        ops[eo4][:],
                                        w2t[:, eo4 * 128:(eo4 + 1) * 128],
                                        m_sb[:, mt, :],
                                        start=(mt == 0),
                                        stop=(mt == 15))
                            for eo4 in range(4):
                                eo = eoh * 4 + eo4
                                if half == 0:
                                    # inner += ops + b2 (per-partition)
                                    nc.vector.scalar_tensor_tensor(
                                        inner[:, eo, :], ops[eo4][:],
                                        b2sb[:, eo:eo + 1], inner[:, eo, :],
                                        AOT.add, AOT.add)
                                else:
                                    nc.vector.tensor_tensor(
                                        inner[:, eo, :], inner[:, eo, :],
                                        ops[eo4][:], AOT.add)
                # x = x + inner  (inner holds x + attn + mlp + b2)
                for e in range(8):
                    nc.vector.tensor_tensor(x[:, e, :], x[:, e, :],
                                            inner[:, e, :], AOT.add)
                    if not (l == N_LAYERS - 1 and DEBUG_X):
                        ln_prep(x, e)

            # ---------------- final LN + unembed ----------------
            if DEBUG_X:
                for e in range(8):
                    xs = stage.tile([128, S], F32, tag="lntmp2", bufs=3)
                    nc.vector.tensor_copy(xs[:], x[:, e, :])
                    nc.sync.dma_start(d_out[e * 128:(e + 1) * 128, :], xs[:])
            else:
                layer_norm(x, gbf)
                with tc.tile_pool(name="wu", bufs=2) as wu, \
                     tc.tile_pool(name="ps_u", bufs=5, space="PSUM") as ps_u:
                    for vg in range(2):
                        upl = [ps_u.tile([128, S], F32, tag="ups", bufs=5,
                                         name=f"ups{_i}") for _i in range(4)]
                        for eh in range(2):
                            uwh = wu.tile([128, 4, 512], BF16, tag="uwh",
                                          bufs=2)
                            for e4 in range(4):
                                e = eh * 4 + e4
                                nc.sync.dma_start(
                                    uwh[:, e4, :],
                                    d_uw[e * 128:(e + 1) * 128,
                                         vg * 512:(vg + 1) * 512])
                            for vo4 in range(4):
                                for e4 in range(4):
                                    e = eh * 4 + e4
                                    nc.tensor.matmul(
                                        upl[vo4][:],
                                        uwh[:, e4, vo4 * 128:(vo4 + 1) * 128],
                                        h[:, e, :], start=(e == 0),
                                        stop=(e == 7))
                        for vo4 in range(4):
                            vo = vg * 4 + vo4
                            lst = stage.tile([128, S], F32, tag="lntmp2",
                                             bufs=3)
                            # logits = ups + ub (per-partition)
                            nc.vector.tensor_scalar_add(
                                lst[:], upl[vo4][:], ubsb[:, vo:vo + 1])
                            nc.sync.dma_start(
                                d_out[vo * 128:(vo + 1) * 128, :], lst[:])

        if HWLOOP > 1:
            rtmp = nc.alloc_registers("reps_reg", mybir.ALL_ENGINES)
            nc.regs_load(rtmp, d_reps[0:1, 0:1])
            rv = nc.snap(rtmp, donate=True, min_val=1, max_val=1 << 20)
            with tc.For_i(0, rv, 1):
                body()
        else:
            body()

        top.close()

    nc.compile()
    return nc


def core_token_idx(c: int) -> np.ndarray:
    p = c % 2
    c0, c1 = QCH[p]
    return np.concatenate([np.arange(c0 * CH, (c0 + 1) * CH),
                           np.arange(c1 * CH, (c1 + 1) * CH)])


def build_masks(parity: int):
    """Multiplicative 0/1 masks in the P-tile layout (bf16)."""
    import ml_dtypes
    k_idx = np.arange(128)
    q_idx = np.arange(CH)

    def blk(g, kt, qs):
        lk = GORDER[g]
        lq = QCH[parity][qs]
        kk = lk * CH + kt * 128 + k_idx[:, None]
        qq = lq * CH + q_idx[None, :]
        return (kk <= qq).astype(np.float32)

    maska = np.stack([blk(0, 0, 0), blk(0, 1, 0), blk(2, 0, 0), blk(2, 1, 0)],
                     axis=1).astype(ml_dtypes.bfloat16)
    maskb = np.concatenate(
        [blk(1, 0, 1), blk(1, 1, 1), blk(3, 0, 1), blk(3, 1, 1)],
        axis=1).astype(ml_dtypes.bfloat16)
    return np.ascontiguousarray(maska), np.ascontiguousarray(maskb)


_NC_CACHE = None


def prepare_in_maps(inputs):
    import ml_dtypes
    BF = ml_dtypes.bfloat16
    toks = np.asarray(inputs["toks"], np.float32)
    pos_W = np.asarray(inputs["pos_W"], np.float32)
    attn_W = np.asarray(inputs["attn_W"], np.float32)

    aw = attn_W.reshape(L, E, H, 3 * C)
    # wqk layout: [L, E, 8, 256]: per head-pair, Q of both heads (128)
    # then K of both heads (128) -- contiguous 1-D stationary slabs
    q = aw[:, :, :, 0:C].reshape(L, E, 8, 2 * C)
    k = aw[:, :, :, C:2 * C].reshape(L, E, 8, 2 * C)
    wqk = np.ascontiguousarray(np.concatenate([q, k], axis=-1)).astype(BF)
    wv = np.ascontiguousarray(aw[:, :, :, 2 * C:]).reshape(L, E, H * C).astype(BF)

    def gbrow(g, b):
        return np.stack([np.asarray(g, np.float32),
                         np.asarray(b, np.float32)], axis=-2).astype(BF)

    shared = {
        "wqk": wqk,
        "wv": wv,
        "w1": np.asarray(inputs["mlp_W1"], np.float32).astype(BF),
        "w2": np.asarray(inputs["mlp_W2"], np.float32).astype(BF),
        "b1": np.ascontiguousarray(inputs["mlp_b1"], np.float32),
        "b2": np.ascontiguousarray(
            np.asarray(inputs["mlp_b2"], np.float32)
            .reshape(L, 8, 128).transpose(0, 2, 1)),
        "gb1": gbrow(inputs["ln1_g"], inputs["ln1_b"]),
        "gb2": gbrow(inputs["ln2_g"], inputs["ln2_b"]),
        "gbf": gbrow(inputs["lnf_g"], inputs["lnf_b"]),
        "tokw": np.asarray(inputs["tok_W"], np.float32).astype(BF),
        "uw": np.asarray(inputs["unembed_W"], np.float32).astype(BF),
        "ub": np.ascontiguousarray(
            np.asarray(inputs["unembed_b"], np.float32)
            .reshape(8, 128).T),
    }
    in_maps = []
    for c in range(NC):
        b, p = c // 2, c % 2
        idx = core_token_idx(c)
        ma, mb = build_masks(p)
        m = dict(shared)
        m["toksT"] = np.ascontiguousarray(toks[b, idx, :].T).astype(BF)
        m["posT"] = np.ascontiguousarray(pos_W[idx, :].T)
        m["maska"] = ma
        m["maskb"] = mb
        if HWLOOP > 1:
            m["reps"] = np.array(
                [[int(os.environ.get("KERNEL_REPS", "1"))]], dtype=np.uint32)
        in_maps.append(m)
    return in_maps


def kernel(**inputs) -> np.ndarray:
    global _NC_CACHE
    if _NC_CACHE is None:
        _NC_CACHE = build_program()
    nc = _NC_CACHE
    in_maps = prepare_in_maps(inputs)

    r = run_bass_kernel_spmd(nc, in_maps, core_ids=list(range(NC)))

    out = np.empty((B, T, V), np.float32)
    for c in range(NC):
        b = c // 2
        idx = core_token_idx(c)
        out[b, idx, :] = r.results[c]["logits"].T
    return out


if __name__ == "__main__":
    print("building program...")
    nc0 = build_program()
    print("built ok")



# revision 43
# speedup vs baseline: 1.0120x; 1.0120x over previous
# Trainium2 Bass kernel for an 8-layer dense transformer (B=4, T=1024,
# V=E=1024, H=16, M=4096), 8-way SPMD across one chip.
#
# Sharding: data-parallel over (batch x 2 interleaved token chunks) ->
# 8 shards of 512 tokens.  Even cores own logical 256-token chunks (0,3)
# of their batch, odd cores own (1,2), which balances causal-attention
# work.  Attention needs the full batch's K/V, which lives on exactly
# one partner core, so K and V are exchanged through PAIR-group
# AllGathers ([[0,1],[2,3],...]) -- 1/7th the traffic of an 8-rank
# gather.  Everything else runs locally with replicated weights.
#
# Layouts: the residual stream is feature-major ([E on partitions,
# tokens free]) so every GEMM consumes the natural row-major weight
# layout with zero transposes.  Scores are key-major so the PV matmul
# is native; the softmax denominator comes out of the same PV matmul
# via a 65th all-ones column appended to V; softmax skips the max
# subtraction (logits are O(1) here).  Causal masking is a per-core
# 0/1 multiplicative bf16 mask so the program is identical on all
# cores; cross-partition LN stats/broadcasts go through tiny
# ones-matmuls on the tensor engine, with the LN gain/bias folded into
# rank-2 outer-product broadcasts.
#
# All GEMM operands are bf16 (weights pre-rounded on the host), which
# halves the weight HBM stream vs fp32 and runs the PE at full rate.
# The whole layer's QKV weights are preloaded into SBUF during the
# previous layer's MLP so the projection phase never waits on HBM, and
# the K allgather is issued right after the K projection so it flies
# under the Q/V (V first, so its allgather hides under Q).
#
# Scheduling: the attention head loop is software-pipelined three deep
# (scores h / PV h-2 / drain h-3) so the in-order PE queue never waits
# on the Act-engine softmax exp (the attention-phase critical path;
# scores land in [128,1024] two-bank PSUM tiles so each head needs only
# 3 wide exps).  LayerNorm statistics are streamed: the producing phase
# (attention for LN2, the W2 half-1 window for the next LN1, embedding
# for layer 0) emits the ones-matmul stat accumulations one tile behind
# production, so at the LN boundary only the short scalar chain and the
# rank-2 broadcast matmuls remain.  K/Q/V projections run as e-major
# waves of 4 PSUM banks that consume the LN normalize stream tile by
# tile.  Weight/bias DMAs are batched into multi-KB strided transfers,
# double-buffered a phase ahead (w1/w2/uw/gb/b), and compute-dependent
# staging DMAs ride the gpsimd DGE queue so they never head-of-line
# block weight prefetches on the SP queue.  Tiny scale-0 exp/sqrt ops
# prefetch the Act function-table swaps off the critical path.

import os
import sys

for _p in ("/opt/trn_rl_repo", "/root/.axon_site/_ro/trn_rl_repo"):
    if _p not in sys.path and os.path.isdir(_p):
        sys.path.insert(0, _p)

import numpy as np

import concourse.bass as bass
import concourse.mybir as mybir
import concourse.tile as tile
from concourse import bacc
from concourse.bass_utils import run_bass_kernel_spmd

B, T, V, E, H, M, L, C = 4, 1024, 1024, 1024, 16, 4096, 8, 64
NC = 8          # cores
S = 512         # tokens per core
CH = 256        # chunk size
EPS = 1e-5
SCALE = 1.0 / 8.0   # 1/sqrt(C)

F32 = mybir.dt.float32
BF16 = mybir.dt.bfloat16
AOT = mybir.AluOpType
AFT = mybir.ActivationFunctionType

N_LAYERS = int(os.environ.get("KERNEL_LAYERS", str(L)))
DEBUG_X = bool(int(os.environ.get("KERNEL_DEBUG_X", "0")))
# Replace collectives with local DMA copies (timing experiments).
FAKE_AG = bool(int(os.environ.get("KERNEL_FAKE_AG", "0")))
# Build single-core (offline timeline-simulation only).
SINGLE = bool(int(os.environ.get("KERNEL_SINGLE", "0")))
# Repeat the computation R times in a hardware loop (timing).
HWLOOP = int(os.environ.get("KERNEL_HWLOOP", "1"))
if SINGLE or HWLOOP > 1:
    FAKE_AG = True

# Pair-group allgather concat order is [even rank, odd rank], so the
# gathered K/V token order is [even core's (0,3) | odd core's (1,2)]:
# gathered slot g -> logical chunk GORDER[g].
GORDER = [0, 3, 1, 2]
# Core parity -> logical chunks of its two local q-slots.
QCH = [(0, 3), (1, 2)]

KBLOB = S * E             # elems per rank K blob (bf16), [8*128, 512]
VBLOB = S * E             # elems per rank V blob (bf16), [512, 1024]

# P-tile column base for each (g, kt) score block.
PCOL = {(0, 0): 0, (0, 1): 512, (2, 0): 1024, (2, 1): 1536,
        (1, 0): 2048, (1, 1): 2304, (3, 0): 2560, (3, 1): 2816}
BLK512 = [(0, 0), (0, 1), (2, 0), (2, 1)]   # N=512 (both q slots)
BLK256 = [(1, 0), (1, 1), (3, 0), (3, 1)]   # N=256 (q slot 1 only)
# PV accumulation order: first/last must be full-width (N=512) blocks.
PV_ORDER = [(0, 0), (1, 0), (1, 1), (3, 0), (3, 1), (0, 1), (2, 0), (2, 1)]


def build_program():
    nc = bacc.Bacc("TRN2", target_bir_lowering=False, debug=False,
                   num_devices=(1 if SINGLE else NC))

    # ---- DRAM I/O ----
    d_toksT = nc.dram_tensor("toksT", [V, S], BF16, kind="ExternalInput")
    d_posT = nc.dram_tensor("posT", [E, S], F32, kind="ExternalInput")
    d_maska = nc.dram_tensor("maska", [128, 4, CH], BF16, kind="ExternalInput")
    d_maskb = nc.dram_tensor("maskb", [128, 4 * CH], BF16, kind="ExternalInput")
    d_reps = (nc.dram_tensor("reps", [1, 1], mybir.dt.uint32,
                             kind="ExternalInput") if HWLOOP > 1 else None)
    d_wqk = nc.dram_tensor("wqk", [L, E, H // 2, 4 * C], BF16, kind="ExternalInput")
    d_wv = nc.dram_tensor("wv", [L, E, H * C], BF16, kind="ExternalInput")
    d_w1 = nc.dram_tensor("w1", [L, E, M], BF16, kind="ExternalInput")
    d_w2 = nc.dram_tensor("w2", [L, M, E], BF16, kind="ExternalInput")
    d_b1 = nc.dram_tensor("b1", [L, M], F32, kind="ExternalInput")
    d_b2 = nc.dram_tensor("b2", [L, 128, 8], F32, kind="ExternalInput")
    d_gb1 = nc.dram_tensor("gb1", [L, 2, E], BF16, kind="ExternalInput")
    d_gb2 = nc.dram_tensor("gb2", [L, 2, E], BF16, kind="ExternalInput")
    d_gbf = nc.dram_tensor("gbf", [2, E], BF16, kind="ExternalInput")
    d_tokw = nc.dram_tensor("tokw", [V, E], BF16, kind="ExternalInput")
    d_uw = nc.dram_tensor("uw", [E, V], BF16, kind="ExternalInput")
    d_ub = nc.dram_tensor("ub", [128, 8], F32, kind="ExternalInput")
    d_out = nc.dram_tensor("logits", [V, S], F32, kind="ExternalOutput")

    with tile.TileContext(nc) as tc:
        from contextlib import ExitStack
        top = ExitStack()
        # carried across phases: (ExitStack, (st_mu, st_msq)) for the LN
        # whose stats were streamed by the producing phase
        lnstate = {}
        pers = top.enter_context(tc.tile_pool(name="pers", bufs=1))
        small = top.enter_context(tc.tile_pool(name="small", bufs=1))
        stage = top.enter_context(tc.tile_pool(name="stage", bufs=2))
        bigpool = top.enter_context(tc.tile_pool(name="bigpool", bufs=1))
        dram = top.enter_context(tc.tile_pool(name="dram", bufs=2, space="DRAM"))
        dram_sh = top.enter_context(tc.tile_pool(name="dram_sh", bufs=2,
                                                 space="DRAM"))

        # ---- persistent SBUF state ----
        x = pers.tile([128, 8, S], F32)        # residual (feature-major)
        inner = pers.tile([128, 8, S], F32)    # x + attn_out (+ mlp out)
        h = pers.tile([128, 8, S], BF16)       # LN output / rounded x
        sq = pers.tile([128, 8, S], BF16)      # squares for LN stats
        qsb = pers.tile([128, 8, S], BF16)     # Q (head-pair-major)
        wqk_sb = pers.tile([128, 8, H // 2, 4 * C], BF16)   # layer QKV weights
        wv_sb = pers.tile([128, 8, H * C], BF16)
        maska = pers.tile([128, 4, CH], BF16)
        maskb = pers.tile([128, 4 * CH], BF16)
        ones_col = pers.tile([128, 1], BF16)   # stats lhsT, carries 1/E
        ones_row = pers.tile([1, 128], BF16)   # denom broadcast lhsT
        eps_col = pers.tile([1, 1], BF16)      # +EPS via stats matmul
        ones_s = pers.tile([1, S], BF16)
        gb1d = pers.tile([2, 2, E], BF16)     # [rows, layer%2, E]
        gb2d = pers.tile([2, 2, E], BF16)
        gbf = pers.tile([2, E], BF16)
        b1d = pers.tile([128, 2, 32], F32)
        b2d = pers.tile([128, 2, 8], F32)
        ubsb = pers.tile([128, 8], F32)
        lnst = small.tile([1, 4 * S], F32)     # mu | mu2 | var | rvar
        rstd_t = small.tile([1, S], BF16)
        nmu_ones = small.tile([2, S], BF16)    # row0: -mu*rstd, row1: 1.0

        actwarm = pers.tile([1, 1], F32)
        nc.vector.memset(actwarm[:], 0.0)
        nc.vector.memset(ones_col[:], 1.0 / E)
        nc.vector.memset(ones_row[:], 1.0)
        nc.vector.memset(eps_col[:], EPS)
        nc.vector.memset(ones_s[:], 1.0)
        # both rows 1.0; each layer_norm overwrites row0 with -mu*rstd
        # (engines cannot address partition base 1, so never write row1)
        nc.vector.memset(nmu_ones[:], 1.0)
        nc.sync.dma_start(maska[:], d_maska[:])
        nc.sync.dma_start(maskb[:], d_maskb[:])
        nc.sync.dma_start(gbf[:], d_gbf[:])
        nc.sync.dma_start(ubsb[:], d_ub[:])

        def ln_prep(src_t, t):
            """Stage tile t of src for LN stats: h = bf16 copy, sq = square.
            Called from the producer loop right after src[:, t, :] is
            written, so the stats matmuls can fire immediately."""
            nc.vector.tensor_copy(h[:, t, :], src_t[:, t, :])
            nc.gpsimd.tensor_tensor(sq[:, t, :], src_t[:, t, :],
                                    src_t[:, t, :], AOT.mult)

        def ln_prep_act(src_t, t):
            """ln_prep on the Act engine — used in the W2 tail where the
            DVE is the serial bottleneck and Act sits idle."""
            nc.scalar.copy(h[:, t, :], src_t[:, t, :])
            nc.scalar.square(sq[:, t, :], src_t[:, t, :])

        from contextlib import contextmanager

        @contextmanager
        def _noscope():
            yield

        def ln_stat_mm(st, t, first=None, last=False):
            """Accumulate LN stats for tile t into PSUM: st = (mu, msq)
            [1,S] tiles.  Emitted (with a one-tile lag) by whatever phase
            produced the tile, so the matmuls hide inside that phase.
            `first` marks the first EMITTED tile of the accumulation; the
            msq group is closed later by the +EPS matmul in ln_finish."""
            st_mu, st_msq = st
            if first is None:
                first = (t == 0)
            nc.tensor.matmul(st_mu[:], ones_col[:], h[:, t, :],
                             start=first, stop=last)
            nc.tensor.matmul(st_msq[:], ones_col[:], sq[:, t, :],
                             start=first, stop=last)

        def ln_open_stats(name):
            es = ExitStack()
            stp = es.enter_context(
                tc.tile_pool(name=name, bufs=1, space="PSUM"))
            st = (stp.tile([1, S], F32, tag="mu", name="st_mu"),
                  stp.tile([1, S], F32, tag="msq", name="st_msq"))
            return es, st

        def ln_finish(src_t, gb_t, st):
            """Chain: eps -> square -> var -> recip -> sqrt -> nmu, then
            per-tile rank-2 broadcasts + normalize into h (bf16).
            h = src*outer(g,rstd) + outer(g,-mu*rstd) + outer(b,1)."""
            st_mu, st_msq = st
            nmu = nmu_ones[0:1, :]
            mu2 = lnst[:, 0:S]
            var = lnst[:, S:2 * S]
            rvar = lnst[:, 2 * S:3 * S]
            # EPS rides the stt (saves the eps-matmul PE stall); only
            # one PSUM operand per DVE op is legal, so mu2 goes via Act
            nc.scalar.square(mu2, st_mu[:])
            nc.vector.scalar_tensor_tensor(var, st_msq[:], EPS, mu2,
                                           AOT.add, AOT.subtract)
            nc.vector.reciprocal(rvar, var)
            with nc.allow_low_precision(reason="bf16 rstd"):
                nc.scalar.sqrt(rstd_t[:], rvar)
                # nmu = (mu * -1) * rstd
                nc.vector.scalar_tensor_tensor(
                    nmu, st_mu[:], -1.0, rstd_t[:], AOT.mult, AOT.mult)
            with tc.tile_pool(name="ps_bc", bufs=1, space="PSUM") as ps_bc:
                for t in range(8):
                    bc1 = ps_bc.tile([128, S], F32, tag="bc", bufs=4,
                                     name="bc1")
                    bc2 = ps_bc.tile([128, S], F32, tag="bc", bufs=4,
                                     name="bc2")
                    gsl = gb_t[0:1, t * 128:(t + 1) * 128]
                    gbsl = gb_t[0:2, t * 128:(t + 1) * 128]
                    nc.tensor.matmul(bc1[:], gsl, rstd_t[:], start=True,
                                     stop=True)
                    nc.tensor.matmul(bc2[:], gbsl, nmu_ones[:], start=True,
                                     stop=True)
                    if t < 5:
                        tmp = stage.tile([128, S], F32, tag="lntmp", bufs=4)
                        nc.vector.tensor_tensor(tmp[:], src_t[:, t, :],
                                                bc1[:], AOT.mult)
                        nc.vector.tensor_tensor(h[:, t, :], tmp[:], bc2[:],
                                                AOT.add)
                    else:
                        # gpsimd cannot read PSUM: stage the broadcasts to
                        # SBUF through the (idle) Act engine, then run the
                        # normalize on the Pool engine alongside the DVE lane
                        bc1c = stage.tile([128, S], BF16, tag="bcc", bufs=4)
                        bc2c = stage.tile([128, S], BF16, tag="bcc2", bufs=4)
                        nc.scalar.copy(bc1c[:], bc1[:])
                        nc.scalar.copy(bc2c[:], bc2[:])
                        tmp = stage.tile([128, S], F32, tag="lntmp", bufs=4)
                        nc.gpsimd.tensor_tensor(tmp[:], src_t[:, t, :],
                                                bc1c[:], AOT.mult)
                        nc.gpsimd.tensor_tensor(h[:, t, :], tmp[:], bc2c[:],
                                                AOT.add)

        def load_qkv_weights(l):
            with nc.named_scope("wload"):
                _load_qkv_weights(l)

        def _load_qkv_weights(l):
            nc.sync.dma_start(
                wqk_sb[:],
                d_wqk[l].rearrange("(e p) hp c -> p e hp c", p=128))
            nc.sync.dma_start(
                wv_sb[:],
                d_wv[l].rearrange("(e p) c -> p e c", p=128))

        def body():
            # ---------------- embedding ----------------
            with nc.named_scope("emb"), \
                 tc.tile_pool(name="emb", bufs=1) as emb, \
                 tc.tile_pool(name="wemb", bufs=2) as wemb, \
                 tc.tile_pool(name="ps_emb", bufs=8, space="PSUM") as ps_emb:
                toksr = emb.tile([128, 8, S], BF16)
                psl = [ps_emb.tile([128, S], F32, tag="embps", bufs=8,
                                   name=f"embps{_i}") for _i in range(8)]
                for v in range(8):
                    nc.sync.dma_start(toksr[:, v, :],
                                      d_toksT[v * 128:(v + 1) * 128, :])
                    tw = wemb.tile([128, 8, 128], BF16, tag="twt", bufs=3)
                    nc.sync.dma_start(tw[:],
                                      d_tokw[v * 128:(v + 1) * 128, :]
                                      .rearrange("p (e c) -> p e c", c=128))
                    for e in range(8):
                        nc.tensor.matmul(psl[e][:], tw[:, e, :],
                                         toksr[:, v, :],
                                         start=(v == 0), stop=(v == 7))
                for e in range(8):
                    nc.scalar.copy(x[:, e, :], psl[e][:])
            with tc.tile_pool(name="embp", bufs=1) as embp:
                possb = embp.tile([128, 8, S], F32)
                for e in range(8):
                    nc.sync.dma_start(possb[:, e, :],
                                      d_posT[e * 128:(e + 1) * 128, :])
                    nc.vector.tensor_tensor(x[:, e, :], x[:, e, :],
                                            possb[:, e, :], AOT.add)
                    ln_prep(x, e)
            load_qkv_weights(0)
            nc.sync.dma_start(gb1d[:, 0, :], d_gb1[0])
            nc.sync.dma_start(gb2d[:, 0, :], d_gb2[0])
            nc.sync.dma_start(b1d[:, 0, :],
                              d_b1[0].rearrange("(o p) -> p o", p=128))
            nc.sync.dma_start(b2d[:, 0, :], d_b2[0])

            # ---------------- layers ----------------
            for l in range(N_LAYERS):
                gb1 = gb1d[:, l % 2, :]
                gb2 = gb2d[:, l % 2, :]
                b1sb = b1d[:, l % 2, :]
                b2sb = b2d[:, l % 2, :]

                agk_in = dram.tile([E, S], BF16, tag="agkin")
                agv_in = dram.tile([S, E], BF16, tag="agvin")
                # pair-group collectives don't support the Shared-output
                # optimization (needs >4 cores) -- Local output is standard
                agk_out = dram_sh.tile([2 * KBLOB], BF16, tag="agkout",
                                       addr_space="Local")
                agv_out = dram_sh.tile([2 * VBLOB], BF16, tag="agvout",
                                       addr_space="Local")

                # ---- LN1 ----
                with nc.named_scope('ln1'):
                    if 'ln1' not in lnstate:      # layer 0: stats inline
                        es0, st0 = ln_open_stats("ps_st1")
                        for t in range(8):
                            ln_stat_mm(st0, t, last=(t == 7))
                        lnstate['ln1'] = (es0, st0)
                    es1, st1 = lnstate.pop('ln1')
                    ln_finish(x, gb1, st1)
                    es1.close()

                # exp-table prefetch: scale=0 keeps the value finite;
                # the act-set load this triggers runs in the QKV window
                # instead of stalling the first real softmax exp
                nc.scalar.activation(actwarm[:], actwarm[:], AFT.Exp,
                                     scale=0.0)
                # ---- K projection first; its pair-allgather flies under
                # the Q and V projections ----
                with tc.tile_pool(name="ps_qkv", bufs=8, space="PSUM") as ps_qkv, \
                     nc.named_scope("qkv"):
                    # e-major waves: each wave's matmuls consume the
                    # LN1 normalize stream tile-by-tile instead of the
                    # whole h tensor at once
                    for w in range(2):
                        kpsl = [ps_qkv.tile([128, S], F32, tag="qkps",
                                            bufs=8, name=f"kps{_i}")
                                for _i in range(4)]
                        for e in range(8):
                            for i in range(4):
                                nc.tensor.matmul(
                                    kpsl[i][:],
                                    wqk_sb[:, e, w * 4 + i, 128:256],
                                    h[:, e, :],
                                    start=(e == 0), stop=(e == 7))
                        for i in range(4):
                            hp = w * 4 + i
                            kst = stage.tile([128, S], BF16, tag="kst",
                                             bufs=5)
                            nc.vector.tensor_copy(kst[:], kpsl[i][:])
                            nc.gpsimd.dma_start(
                                agk_in[hp * 128:(hp + 1) * 128, :], kst[:])
                    if FAKE_AG:
                        nc.gpsimd.dma_start(
                            agk_out[0:KBLOB]
                            .rearrange("(r c) -> r c", c=S)[0:128, :],
                            agk_in[0:128, :])
                        nc.gpsimd.dma_start(
                            agk_out[KBLOB:2 * KBLOB]
                            .rearrange("(r c) -> r c", c=S)[0:128, :],
                            agk_in[0:128, :])
                    else:
                        nc.gpsimd.collective_compute(
                            "AllGather", AOT.bypass,
                            replica_groups=[[2 * i, 2 * i + 1]
                                            for i in range(NC // 2)],
                            ins=[agk_in[:].rearrange("r c -> (r c)").opt()],
                            outs=[agk_out[:].opt()])
                    # ---- V projection (token-major) ----
                    for hf in range(2):
                        vpsl = [ps_qkv.tile([128, S], F32, tag="qkps",
                                            bufs=8, name=f"vps{_i}")
                                for _i in range(4)]
                        for e in range(8):
                            for tt in range(4):
                                nc.tensor.matmul(
                                    vpsl[tt][:],
                                    h[:, e, tt * 128:(tt + 1) * 128],
                                    wv_sb[:, e, hf * 512:(hf + 1) * 512],
                                    start=(e == 0), stop=(e == 7))
                        for tt in range(4):
                            vst = stage.tile([128, S], BF16, tag="vst",
                                             bufs=5)
                            nc.vector.tensor_copy(vst[:], vpsl[tt][:])
                            nc.gpsimd.dma_start(
                                agv_in[tt * 128:(tt + 1) * 128,
                                       hf * 512:(hf + 1) * 512], vst[:])
                    if FAKE_AG:
                        nc.gpsimd.dma_start(
                            agv_out[0:VBLOB]
                            .rearrange("(r c) -> r c", c=E)[0:64, :],
                            agv_in[0:64, :])
                        nc.gpsimd.dma_start(
                            agv_out[VBLOB:2 * VBLOB]
                            .rearrange("(r c) -> r c", c=E)[0:64, :],
                            agv_in[0:64, :])
                    else:
                        nc.gpsimd.collective_compute(
                            "AllGather", AOT.bypass,
                            replica_groups=[[2 * i, 2 * i + 1]
                                            for i in range(NC // 2)],
                            ins=[agv_in[:].rearrange("r c -> (r c)").opt()],
                            outs=[agv_out[:].opt()])

                    # ---- Q projection ----
                    for w in range(2):
                        qpsl = [ps_qkv.tile([128, S], F32, tag="qkps",
                                            bufs=8, name=f"qps{_i}")
                                for _i in range(4)]
                        for e in range(8):
                            for i in range(4):
                                nc.tensor.matmul(
                                    qpsl[i][:],
                                    wqk_sb[:, e, w * 4 + i, 0:128],
                                    h[:, e, :],
                                    start=(e == 0), stop=(e == 7))
                        for i in range(4):
                            nc.vector.tensor_copy(qsb[:, w * 4 + i, :],
                                                  qpsl[i][:])
                # gathered V (token-major, with interleaved 1/1 column for
                # the softmax denominator)
                vgr = bigpool.tile([128, 8, H, C + 1], BF16, tag="vgrm",
                                   bufs=1)
                for kt in range(8):
                    off = (0 if kt < 4 else VBLOB) + (kt % 4) * 128 * E
                    vsrc = agv_out[bass.ds(off, 128 * E)].rearrange(
                        "(r hh cc) -> r hh cc", hh=H, cc=C)
                    nc.sync.dma_start(vgr[:, kt, :, 0:C], vsrc)
                nc.vector.memset(vgr[:, :, :, C:C + 1], 1.0)

                # ---- attention ----
                # LN2 stats accumulate during attention
                st2_es, st2 = ln_open_stats("ps_st2")
                # Software-pipelined head loop: the PE stream is
                # S(h), V(h-1), D(h-2) so no PE instruction ever sits at
                # the head of the queue waiting on the Act exp (V needs
                # exp+mask of its own head) or the DVE reciprocal (D
                # needs it for the 1/denom broadcast).  Act (softmax exp,
                # ~3.1us/head) is the attention-phase critical path; the
                # PE just has to stay out of its way.
                with nc.named_scope("attn"), \
                     tc.tile_pool(name="attnp", bufs=1) as attnp, \
                     tc.tile_pool(name="ps_s", bufs=1, space="PSUM") as ps_s, \
                     tc.tile_pool(name="ps_y", bufs=1, space="PSUM") as ps_y:
                    krs = {}

                    def kr_fetch(hp):
                        if hp > 7 or hp in krs:
                            return
                        kr = attnp.tile([128, 1024], BF16, tag="kr",
                                        bufs=3, name="kr")
                        nc.sync.dma_start(
                            kr[:].rearrange("r (b c) -> r b c", b=2),
                            agk_out[:]
                            .rearrange("(b h r c) -> r h b c", b=2, h=8,
                                       c=S)[:, hp, :, :])
                        krs[hp] = kr

                    def stage_s(ha):
                        hp, head = divmod(ha, 2)
                        qb = head * 64
                        if head == 0:
                            kr_fetch(hp)
                            kr_fetch(hp + 1)   # prefetch next pair's K
                        kr = krs[hp]
                        P = attnp.tile([128, 3072], BF16, tag="P", bufs=3,
                                       name="P")
                        for pair in range(2):
                            sps = ps_s.tile([128, 1024], F32, tag="S",
                                            bufs=2, name="sps")
                            for kt in range(2):
                                g = pair * 2      # g = 0 then 2
                                ktg = g * 2 + kt
                                nc.tensor.matmul(
                                    sps[:, kt * 512:(kt + 1) * 512],
                                    kr[qb:qb + 64,
                                       ktg * 128:(ktg + 1) * 128],
                                    qsb[qb:qb + 64, hp, :],
                                    start=True, stop=True)
                            pc = PCOL[(g, 0)]
                            nc.scalar.activation(P[:, pc:pc + 1024],
                                                 sps[:], AFT.Exp,
                                                 scale=SCALE)
                        sps = ps_s.tile([128, 1024], F32, tag="S",
                                        bufs=2, name="sps")
                        for gi, g in enumerate((1, 3)):
                            for kt in range(2):
                                ktg = g * 2 + kt
                                o = gi * 512 + kt * 256
                                nc.tensor.matmul(
                                    sps[:, o:o + 256],
                                    kr[qb:qb + 64,
                                       ktg * 128:(ktg + 1) * 128],
                                    qsb[qb:qb + 64, hp, 256:512],
                                    start=True, stop=True)
                        nc.scalar.activation(P[:, 2048:3072], sps[:],
                                             AFT.Exp, scale=SCALE)
                        return P

                    def stage_v(ha, P):
                        hp, head = divmod(ha, 2)
                        # causal mask (0/1 multiplicative).  Applied here,
                        # two pipeline stages after the exps were issued,
                        # so the DVE never sits at the head of its queue
                        # waiting for an exp that hasn't run yet.
                        ap1 = P[:, 0:2048].rearrange(
                            "p (b q) -> p b q", q=512)[:, :, 0:CH]
                        nc.vector.tensor_tensor(ap1, ap1, maska[:],
                                                AOT.mult)
                        ap2 = P[:, 2048:3072]
                        nc.vector.tensor_tensor(ap2, ap2, maskb[:],
                                                AOT.mult)
                        yps = ps_y.tile([128, 512], F32, tag="y", bufs=2,
                                        name="yps")
                        for i, (g, kt) in enumerate(PV_ORDER):
                            ktg = g * 2 + kt
                            pc = PCOL[(g, kt)]
                            n = 512 if (g, kt) in BLK512 else 256
                            qoff = 0 if n == 512 else 256
                            nc.tensor.matmul(
                                yps[0:65, qoff:qoff + n],
                                vgr[:, ktg, ha, :],
                                P[:, pc:pc + n],
                                start=(i == 0),
                                stop=(i == len(PV_ORDER) - 1))
                        rd = stage.tile([1, S], BF16, tag="rd", bufs=3)
                        with nc.allow_low_precision(reason="1/d bcast"):
                            nc.vector.reciprocal(rd[:], yps[64:65, :])
                        ebase = head * 64
                        ysb = stage.tile([128, S], BF16, tag="ysb", bufs=2)
                        ysl = ysb[ebase:ebase + 64, :]
                        nc.vector.tensor_copy(ysl, yps[0:64, :])
                        return rd, ysl

                    def stage_d(ha, rd, ysl):
                        hp, head = divmod(ha, 2)
                        # 1/denom broadcast matmul: by this stage the
                        # reciprocal is three pipeline steps old, so the
                        # PE never waits on the DVE here
                        bcd = ps_y.tile([64, S], F32, tag="y", bufs=2,
                                        name="bcd")
                        nc.tensor.matmul(bcd[:], ones_row[:, 0:64],
                                         rd[:], start=True, stop=True)
                        nc.vector.tensor_tensor(ysl, ysl, bcd[:],
                                                AOT.mult)
                        # inner = x + y   (e-tile == hp)
                        ebase = head * 64
                        nc.gpsimd.tensor_tensor(
                            inner[ebase:ebase + 64, hp, :],
                            x[ebase:ebase + 64, hp, :], ysl, AOT.add)
                        if head == 1:
                            ln_prep(inner, hp)
                            # stats for the PREVIOUS pair: its ln_prep
                            # outputs are long since ready, so the PE
                            # never stalls at the head of its queue here
                            if hp > 1:
                                ln_stat_mm(st2, hp - 2, first=(hp == 2))

                    pipe = []
                    for ha in range(16):
                        pipe.append((ha, stage_s(ha)))
                        if len(pipe) >= 3:
                            hv, Pv = pipe[-3]
                            pipe[-3] = (hv, stage_v(hv, Pv))
                        if len(pipe) >= 4:
                            hd, (rdv, ylv) = pipe.pop(0)
                            stage_d(hd, rdv, ylv)
                    for k in (-2, -1):
                        hv, Pv = pipe[k]
                        pipe[k] = (hv, stage_v(hv, Pv))
                    for hd, (rdv, ylv) in pipe:
                        stage_d(hd, rdv, ylv)
                    ln_stat_mm(st2, 6, first=False)
                    ln_stat_mm(st2, 7, first=False, last=True)
                    # sqrt-table prefetch for the LN2 chain
                    nc.scalar.sqrt(actwarm[:], actwarm[:])
                # ---- LN2 + MLP (mlp out accumulates into `inner`) ----
                with tc.tile_pool(name="wmlp", bufs=1) as wmlp:
                    # first W1 weight tiles stream in under LN2-finish
                    w1pre = []
                    for eh in range(2):
                        w1p = wmlp.tile([128, 4, 512], BF16, tag="w1h",
                                        bufs=3, name="w1h")
                        nc.sync.dma_start(
                            w1p[:],
                            d_w1[l, eh * 512:(eh + 1) * 512, 0:512]
                            .rearrange("(e p) c -> p e c", p=128))
                        w1pre.append(w1p)
                    with nc.named_scope('ln2'):
                        ln_finish(inner, gb2, st2)
                        st2_es.close()
                    if l + 1 < N_LAYERS:
                        nc.sync.dma_start(gb1d[:, (l + 1) % 2, :],
                                          d_gb1[l + 1])
                        nc.sync.dma_start(gb2d[:, (l + 1) % 2, :],
                                          d_gb2[l + 1])
                        nc.sync.dma_start(b1d[:, (l + 1) % 2, :],
                                          d_b1[l + 1].rearrange(
                                              "(o p) -> p o", p=128))
                        nc.sync.dma_start(b2d[:, (l + 1) % 2, :],
                                          d_b2[l + 1])
                    for half in range(2):
                        m_sb = bigpool.tile([128, 16, 512], BF16, tag="vgrm",
                                            bufs=1)
                        nc.enter_named_scope("w1", False)
                        ps_m_es = ExitStack()
                        ps_m = ps_m_es.enter_context(
                            tc.tile_pool(name="ps_m", bufs=1, space="PSUM"))
                        for og4 in range(4):
                            og = half * 4 + og4
                            mps_l = [ps_m.tile([128, S], F32, tag="mps",
                                               bufs=4, name=f"mps{_i}")
                                     for _i in range(4)]
                            for eh in range(2):
                                if half == 0 and og4 == 0:
                                    w1h = w1pre[eh]
                                else:
                                    w1h = wmlp.tile([128, 4, 512], BF16,
                                                    tag="w1h", bufs=3,
                                                    name="w1h")
                                    nc.sync.dma_start(
                                        w1h[:],
                                        d_w1[l, eh * 512:(eh + 1) * 512,
                                             og * 512:(og + 1) * 512]
                                        .rearrange("(e p) c -> p e c",
                                                   p=128))
                                for ob in range(4):
                                    for e4 in range(4):
                                        e = eh * 4 + e4
                                        nc.tensor.matmul(
                                            mps_l[ob][:],
                                            w1h[:, e4,
                                                ob * 128:(ob + 1) * 128],
                                            h[:, e, :],
                                            start=(e == 0), stop=(e == 7))
                            for ob in range(4):
                                mtl = og4 * 4 + ob
                                mt_abs = half * 16 + mtl
                                nc.scalar.activation(
                                    m_sb[:, mtl, :], mps_l[ob][:], AFT.Relu,
                                    bias=b1sb[:, mt_abs:mt_abs + 1], scale=1.0)
                        ps_m_es.close()
                        nc.leave_named_scope("w1", 0, False)
                        # next layer's QKV weights stream during the W2
                        # windows (half each, to spread HBM pressure)
                        if l + 1 < N_LAYERS:
                            if half == 0:
                                nc.sync.dma_start(
                                    wqk_sb[:],
                                    d_wqk[l + 1].rearrange(
                                        "(e p) hp c -> p e hp c", p=128))
                            else:
                                nc.sync.dma_start(
                                    wv_sb[:],
                                    d_wv[l + 1].rearrange(
                                        "(e p) c -> p e c", p=128))
                        stream_ln = (half == 1 and
                                     not (l == N_LAYERS - 1 and DEBUG_X))
                        if stream_ln:
                            # the *next* LN1's stats accumulate as x tiles
                            # finalize inside this W2 window
                            es_n, st_n = ln_open_stats("ps_st1")
                        # first W2 weight tile streams in under the last
                        # W1 group's relu drain
                        w2pre = wmlp.tile([128, 4, 512], BF16, tag="w2",
                                          bufs=2, name="w2q")
                        mt0p = half * 16
                        nc.sync.dma_start(
                            w2pre[:],
                            d_w2[l, mt0p * 128:(mt0p + 4) * 128, 512:1024]
                            .rearrange("(m p) c -> p m c", p=128))
                        nc.enter_named_scope("w2", False)
                        ps_o_es = ExitStack()
                        ps_o = ps_o_es.enter_context(
                            tc.tile_pool(name="ps_o", bufs=1, space="PSUM"))
                        stat_pend = []
                        st_emit = [0]
                        for eoh in (1, 0):
                            ops = [ps_o.tile([128, S], F32, tag="ops", bufs=4,
                                             name=f"ops{_i}")
                                   for _i in range(4)]
                            for mq in range(4):
                                if mq == 0 and eoh == 1:
                                    w2q = w2pre
                                else:
                                    w2q = wmlp.tile([128, 4, 512], BF16,
                                                    tag="w2", bufs=2,
                                                    name="w2q")
                                    mt0 = half * 16 + mq * 4
                                    nc.sync.dma_start(
                                        w2q[:],
                                        d_w2[l, mt0 * 128:(mt0 + 4) * 128,
                                             eoh * 512:(eoh + 1) * 512]
                                        .rearrange("(m p) c -> p m c",
                                                   p=128))
                                for m4 in range(4):
                                    mt = mq * 4 + m4
                                    for eo4 in range(4):
                                        nc.tensor.matmul(
                                            ops[eo4][:],
                                            w2q[:, m4,
                                                eo4 * 128:(eo4 + 1) * 128],
                                            m_sb[:, mt, :],
                                            start=(mt == 0),
                                            stop=(mt == 15))
                                if stat_pend:
                                    # a pending LN-stats matmul, spaced out
                                    # so its ln_prep inputs are long ready
                                    t0 = stat_pend.pop(0)
                                    ln_stat_mm(st_n, t0,
                                               first=(st_emit[0] == 0))
                                    st_emit[0] += 1
                            for eo4 in range(4):
                                eo = eoh * 4 + eo4
                                if half == 0:
                                    # inner += ops + b2 (per-partition)
                                    nc.vector.scalar_tensor_tensor(
                                        inner[:, eo, :], ops[eo4][:],
                                        b2sb[:, eo:eo + 1], inner[:, eo, :],
                                        AOT.add, AOT.add)
                                else:
                                    # inner += ops, then finalize the
                                    # residual x tile; the next LN's
                                    # stats are emitted one tile late so
                                    # the PE never waits on this tile's
                                    # ln_prep chain
                                    nc.vector.tensor_tensor(
                                        inner[:, eo, :], inner[:, eo, :],
                                        ops[eo4][:], AOT.add)
                                    nc.vector.tensor_tensor(
                                        x[:, eo, :], x[:, eo, :],
                                        inner[:, eo, :], AOT.add)
                                    if stream_ln:
                                        ln_prep_act(x, eo)
                                        stat_pend.append(eo)
                        ps_o_es.close()
                        nc.leave_named_scope("w2", 0, False)
                    if stream_ln:
                        while stat_pend:
                            t0 = stat_pend.pop(0)
                            ln_stat_mm(st_n, t0, first=(st_emit[0] == 0),
                                       last=not stat_pend)
                            st_emit[0] += 1
                        lnstate['ln1'] = (es_n, st_n)

            # ---------------- final LN + unembed ----------------
            if DEBUG_X:
                for e in range(8):
                    xs = stage.tile([128, S], F32, tag="lntmp", bufs=4)
                    nc.vector.tensor_copy(xs[:], x[:, e, :])
                    nc.sync.dma_start(d_out[e * 128:(e + 1) * 128, :], xs[:])
            else:
                with nc.named_scope('lnf'):
                    esf, stf = lnstate.pop('ln1')
                    ln_finish(x, gbf, stf)
                    esf.close()
                with nc.named_scope("unemb"), \
                     tc.tile_pool(name="wu", bufs=2) as wu, \
                     tc.tile_pool(name="ps_u", bufs=5, space="PSUM") as ps_u:
                    for vg in range(2):
                        upl = [ps_u.tile([128, S], F32, tag="ups", bufs=5,
                                         name=f"ups{_i}") for _i in range(4)]
                        for eh in range(2):
                            uwh = wu.tile([128, 4, 512], BF16, tag="uwh",
                                          bufs=2)
                            nc.sync.dma_start(
                                uwh[:],
                                d_uw[eh * 512:(eh + 1) * 512,
                                     vg * 512:(vg + 1) * 512]
                                .rearrange("(e p) c -> p e c", p=128))
                            for vo4 in range(4):
                                for e4 in range(4):
                                    e = eh * 4 + e4
                                    nc.tensor.matmul(
                                        upl[vo4][:],
                                        uwh[:, e4, vo4 * 128:(vo4 + 1) * 128],
                                        h[:, e, :], start=(e == 0),
                                        stop=(e == 7))
                        for vo4 in range(4):
                            vo = vg * 4 + vo4
                            lst = stage.tile([128, S], F32, tag="lntmp",
                                             bufs=4)
                            # logits = ups + ub (per-partition)
                            nc.vector.tensor_scalar_add(
                                lst[:], upl[vo4][:], ubsb[:, vo:vo + 1])
                            nc.scalar.dma_start(
                                d_out[vo * 128:(vo + 1) * 128, :], lst[:])

        if HWLOOP > 1:
            rtmp = nc.alloc_registers("reps_reg", mybir.ALL_ENGINES)
            nc.regs_load(rtmp, d_reps[0:1, 0:1])
            rv = nc.snap(rtmp, donate=True, min_val=1, max_val=1 << 20)
            with tc.For_i(0, rv, 1):
                body()
        else:
            body()

        top.close()

    nc.compile()
    return nc


def core_token_idx(c: int) -> np.ndarray:
    p = c % 2
    c0, c1 = QCH[p]
    return np.concatenate([np.arange(c0 * CH, (c0 + 1) * CH),
                           np.arange(c1 * CH, (c1 + 1) * CH)])


def build_masks(parity: int):
    """Multiplicative 0/1 masks in the P-tile layout (bf16)."""
    import ml_dtypes
    k_idx = np.arange(128)
    q_idx = np.arange(CH)

    def blk(g, kt, qs):
        lk = GORDER[g]
        lq = QCH[parity][qs]
        kk = lk * CH + kt * 128 + k_idx[:, None]
        qq = lq * CH + q_idx[None, :]
        return (kk <= qq).astype(np.float32)

    maska = np.stack([blk(0, 0, 0), blk(0, 1, 0), blk(2, 0, 0), blk(2, 1, 0)],
                     axis=1).astype(ml_dtypes.bfloat16)
    maskb = np.concatenate(
        [blk(1, 0, 1), blk(1, 1, 1), blk(3, 0, 1), blk(3, 1, 1)],
        axis=1).astype(ml_dtypes.bfloat16)
    return np.ascontiguousarray(maska), np.ascontiguousarray(maskb)


_NC_CACHE = None


def prepare_in_maps(inputs):
    import ml_dtypes
    BF = ml_dtypes.bfloat16
    toks = np.asarray(inputs["toks"], np.float32)
    pos_W = np.asarray(inputs["pos_W"], np.float32)
    attn_W = np.asarray(inputs["attn_W"], np.float32)

    aw = attn_W.reshape(L, E, H, 3 * C)
    # wqk layout: [L, E, 8, 256]: per head-pair, Q of both heads (128)
    # then K of both heads (128) -- contiguous 1-D stationary slabs
    q = aw[:, :, :, 0:C].reshape(L, E, 8, 2 * C)
    k = aw[:, :, :, C:2 * C].reshape(L, E, 8, 2 * C)
    wqk = np.ascontiguousarray(np.concatenate([q, k], axis=-1)).astype(BF)
    wv = np.ascontiguousarray(aw[:, :, :, 2 * C:]).reshape(L, E, H * C).astype(BF)

    def gbrow(g, b):
        return np.stack([np.asarray(g, np.float32),
                         np.asarray(b, np.float32)], axis=-2).astype(BF)

    shared = {
        "wqk": wqk,
        "wv": wv,
        "w1": np.asarray(inputs["mlp_W1"], np.float32).astype(BF),
        "w2": np.asarray(inputs["mlp_W2"], np.float32).astype(BF),
        "b1": np.ascontiguousarray(inputs["mlp_b1"], np.float32),
        "b2": np.ascontiguousarray(
            np.asarray(inputs["mlp_b2"], np.float32)
            .reshape(L, 8, 128).transpose(0, 2, 1)),
        "gb1": gbrow(inputs["ln1_g"], inputs["ln1_b"]),
        "gb2": gbrow(inputs["ln2_g"], inputs["ln2_b"]),
        "gbf": gbrow(inputs["lnf_g"], inputs["lnf_b"]),
        "tokw": np.asarray(inputs["tok_W"], np.float32).astype(BF),
        "uw": np.asarray(inputs["unembed_W"], np.float32).astype(BF),
        "ub": np.ascontiguousarray(
            np.asarray(inputs["unembed_b"], np.float32)
            .reshape(8, 128).T),
    }
    in_maps = []
    for c in range(NC):
        b, p = c // 2, c % 2
        idx = core_token_idx(c)
        ma, mb = build_masks(p)
        m = dict(shared)
        m["toksT"] = np.ascontiguousarray(toks[b, idx, :].T).astype(BF)
        m["posT"] = np.ascontiguousarray(pos_W[idx, :].T)
        m["maska"] = ma
        m["maskb"] = mb
        if HWLOOP > 1:
            m["reps"] = np.array(
                [[int(os.environ.get("KERNEL_REPS", "1"))]], dtype=np.uint32)
        in_maps.append(m)
    return in_maps


def kernel(**inputs) -> np.ndarray:
    global _NC_CACHE
    if _NC_CACHE is None:
        _NC_CACHE = build_program()
    nc = _NC_CACHE
    in_maps = prepare_in_maps(inputs)

    r = run_bass_kernel_spmd(nc, in_maps, core_ids=list(range(NC)))

    out = np.empty((B, T, V), np.float32)
    for c in range(NC):
        b = c // 2
        idx = core_token_idx(c)
        out[b, idx, :] = r.results[c]["logits"].T
    return out


if __name__ == "__main__":
    print("building program...")
    nc0 = build_program()
    print("built ok")

